# revision 1
# baseline (speedup 1.0000x reference)
"""Trainium2 Bass kernel for nn_LossFunction_29145648071076.

Math notes (verified against the reference in float64):

  * Q = x x^H is rank-1 (x = sum of comm + sensing beams), so
      gHQg[b,l]  = |DUMatInit[b,l]^H x_b|^2
      P[b,g]     = |a_g^H x_b|^2
    and no NTxNT matrices are ever needed.

  * The uplink MMSE path collapses exactly: A = D - p_k u_k u_k^H differs
    from D by rank-1, so w = A^{-1}u is a scalar multiple of D^{-1}u and
    num/den == p_k c_k with c_k = u_k^H D^{-1} u_k.  With D = sum_j p_j
    u_j u_j^H + v v^H + nBS*I and nBS = 1e-9, Woodbury gives
    p_k c_k = 1 - nBS*[M^{-1}]_kk = 1 - O(1e-7), hence
    sum_rate_uu = K = 16 to within 1e-7 bits (2.5e-14 relative effect on
    the ~2.58e6 loss, which the beampattern term dominates).  The kernel
    uses the constant.

  * nDU = 10^(noise2DU/10) = 1e-9 added to a denominator that is ~21;
    the effect is below one f32 ulp of the result (<1e-10 relative), so
    the term is dropped on device.

  * Data parallel over the batch: B=128 split 16 samples per core across
    8 NeuronCores; each core emits (sum_s sum_g diff^2, sum_{s,l}
    ln(1+r)) and the host gathers/means the 8 partial scalars.
"""

import numpy as np

B, NT, NR, K, L, M, I = 128, 64, 64, 16, 16, 8, 8
NCORES = 8
S = B // NCORES          # samples per core
G = 181                  # beampattern grid points
LN2 = float(np.log(2.0))

ROWS_W = S * 48          # 768
DUMT_W = S * 32          # 512
AG_W = 4 * G             # 724: [ar | ai | ai | -ar]

NWARM = 6
_CACHE = {}


def _steering_consts():
    """a_g table computed with the reference's f32 rounding order."""
    grid = np.linspace(0.0, 180.0, G).astype(np.float32)
    n = np.arange(NT, dtype=np.float32)
    sin_t = np.sin(grid * np.float32(np.pi / 180.0)).astype(np.float32)
    phase = (np.float32(np.pi) * sin_t)[:, None] * n          # (G, NT) f32
    ar = np.cos(phase).astype(np.float32).T                   # (NT, G)
    ai = np.sin(phase).astype(np.float32).T
    agT = np.concatenate([ar, ai, ai, -ar], axis=1).astype(np.float32)
    return np.ascontiguousarray(agT)                          # (64, 4G)


def _emit_body(nc, tc, sb, ps, d, mybir, warm=True):
    """Emit one kernel body. Tile tags come from variable names, so
    re-emitting with the same pool serializes replicas via slot reuse
    (used by the benchmark)."""
    import concourse.bass as bass

    AF = mybir.ActivationFunctionType
    OP = mybir.AluOpType
    AX = mybir.AxisListType
    f32 = mybir.dt.float32
    bf16 = mybir.dt.bfloat16

    # Dummy Ln first: loads the natural_log act table at t~0 (ACT
    # is idle), and that table also serves Abs/Sign/Square/Copy —
    # so no further table load lands on the critical path.
    t_dl = sb.tile([1, 1], f32)
    nc.vector.memset(t_dl[:], 0.0)
    nc.scalar.activation(t_dl[:], t_dl[:], AF.Ln, bias=1.0)

    # ---- loads, most-urgent first; b32 early so the nuu/CI path
    # (which feeds the serial downlink tail) is never DMA-gated ----
    t_rows = sb.tile([64, ROWS_W], f32)
    nc.sync.dma_start(t_rows[:, 0:ROWS_W // 2], d["rows0"][:])
    nc.sync.dma_start(t_rows[:, ROWS_W // 2:], d["rows1"][:])
    t_ag = sb.tile([64, AG_W], f32)
    nc.sync.dma_start(t_ag[:], d["agt"][:])
    t_128 = sb.tile([128, 17], f32)         # [-taang | blk(16)]
    nc.sync.dma_start(t_128[:], d["b128"][:])
    t_dm = sb.tile([64, DUMT_W], f32)
    nc.sync.dma_start(t_dm[:], d["dumt"][:])
    t_32 = sb.tile([32, 272], f32)          # [cicat | pmat]
    nc.sync.dma_start(t_32[:], d["b32"][:])

    t_ta = t_128[:, 0:1]
    t_blk = t_128[:, 1:17]
    t_ci = t_32[:, 0:256]
    t_pm = t_32[:, 256:272]

    # ---- x = row-sums: (64, S,2,24) -> Xcat (64, 2S) ----
    t_x = sb.tile([64, 2 * S], f32)
    rows_v = t_rows[:].rearrange("p (a j) -> p a j", j=24)
    nc.vector.tensor_reduce(t_x[:, 0:S], rows_v[:, 0:S, :],
                            axis=AX.X, op=OP.add)
    nc.vector.tensor_reduce(t_x[:, S:2 * S], rows_v[:, S:2 * S, :],
                            axis=AX.X, op=OP.add)
    xv = t_x[:].rearrange("p (s c) -> p s c", c=2)

    # Xalt: even cols = xi_s, odd cols = -xr_s
    t_xa = sb.tile([64, 2 * S], f32)
    xav = t_xa[:].rearrange("p (s c) -> p s c", c=2)
    nc.vector.tensor_copy(xav[:, :, 0:1], xv[:, :, 1:2])
    nc.vector.tensor_scalar_mul(xav[:, :, 1:2], xv[:, :, 0:1], -1.0)
    Xr = xv[:, :, 0]
    Xi = xv[:, :, 1]

    # ---- PE p-state warmup: keep the tensor engine busy from
    # t~0 so the clock is fully ramped (2.4 GHz vs 1.2) when the
    # real matmuls arrive.  Constant inputs, scratch PSUM bank.
    if warm:
        t_wsrc = sb.tile([64, 512], bf16)
        nc.gpsimd.memset(t_wsrc[:], 0.0)
        p_warm_b = ps.tile([1, 512], f32)
        for _ in range(NWARM):
            nc.tensor.matmul(p_warm_b[:], t_wsrc[:, 0:1], t_wsrc[:])

    # ---- [Re | Im] of a_g^H x as (S, 2G): 2 f32 matmuls ----
    # (f32r would be 4x faster on the PE but is TF32-like
    # (~1.4e-4 rel err, measured); plain f32 keeps the result
    # bit-exact vs the reference.)
    p_ri_b = ps.tile([16, 512], f32)
    p_ri = p_ri_b[:, 0:2 * G]
    nc.tensor.matmul(p_ri, Xr, t_ag[:, 0:2 * G],
                     start=True, stop=False)
    nc.tensor.matmul(p_ri, Xi, t_ag[:, 2 * G:4 * G],
                     start=False, stop=True)

    # ---- gx = DUMat^H x per sample (PE, right after P) ----
    p_gx_b = ps.tile([16, 512], f32)
    p_gx = p_gx_b[:, 0:4 * S]
    for s in range(S):
        nc.tensor.matmul(
            p_gx[:, 4 * s:4 * s + 2],
            t_dm[:, 32 * s:32 * s + 16],
            t_x[:, 2 * s:2 * s + 2])
        nc.tensor.matmul(
            p_gx[:, 4 * s + 2:4 * s + 4],
            t_dm[:, 32 * s + 16:32 * s + 32],
            t_x[:, 2 * s:2 * s + 2])
    t_gxs = sb.tile([16, 4 * S], f32)
    nc.scalar.copy(t_gxs[:], p_gx)
    t_cis = sb.tile([32, 256], f32)
    nc.scalar.activation(t_cis[:], t_ci, AF.Square)

    # ---- mask: b_theta (S, G); grid 0..180 via f32 iota ----
    # |g - ta| on ACT (Abs, bias = -ta), sign(10 - d) in {-1,+1}
    # as bf16 (exact for 0/+-1), bf16 count matmul (exact, count
    # <= 8), "any in range" == count >= -7.
    t_grid = sb.tile([128, G], f32)
    nc.gpsimd.iota(t_grid[:], [[1, G]], channel_multiplier=0,
                   allow_small_or_imprecise_dtypes=True)
    t_d = sb.tile([128, G], f32)
    nc.scalar.activation(t_d[:], t_grid[:], AF.Abs, bias=t_ta)
    t_ind = sb.tile([128, G], bf16)
    nc.vector.tensor_scalar(t_ind[:], t_d[:], 10.0, None,
                            op0=OP.is_le)
    t_blkb = sb.tile([128, 16], bf16)
    nc.vector.tensor_copy(t_blkb[:], t_blk)
    p_cnt_b = ps.tile([16, 512], f32)
    p_cnt = p_cnt_b[:, 0:G]
    nc.tensor.matmul(p_cnt, t_blkb[:], t_ind[:])

    # ---- noiseUU2DU matmuls; |CI|^2 prep on Pool (keeps the PE
    # wait on a quiet semaphore stream) ----
    t_ci2 = sb.tile([32, 128], f32)
    civ = t_cis[:].rearrange("p (j c l) -> p j c l", j=8, c=2)
    ci2o = t_ci2[:].rearrange("p (j l) -> p j l", j=8)
    nc.gpsimd.tensor_add(ci2o[:], civ[:, :, 0, :], civ[:, :, 1, :])
    p_nu_b = ps.tile([16, 512], f32)
    p_nu = p_nu_b[:, 0:16]
    for j in range(8):
        nc.tensor.matmul(
            p_nu[:, 2 * j:2 * j + 2],
            t_ci2[:, 16 * j:16 * j + 16],
            t_pm[:, 2 * j:2 * j + 2])
    t_fin = sb.tile([16, 2], f32)
    # ---- beampattern loss: sum diff^2 == sum P^2 - bp^2/bb ----
    # (diff = beta*b - P, beta = bp/bb; b in {0,1} collapses the
    # cross terms; no catastrophic cancellation: bp^2/bb is ~16%
    # of sum P^2 on this data.)
    t_p1 = sb.tile([16, G], f32)
    nc.scalar.activation(t_p1[:], p_ri[:, 0:G], AF.Square)
    t_p2 = sb.tile([16, G], f32)
    nc.scalar.activation(t_p2[:], p_ri[:, G:2 * G], AF.Square)
    t_pp = sb.tile([16, G], f32)
    nc.vector.tensor_add(t_pp[:], t_p1[:], t_p2[:])
    t_b = sb.tile([16, G], f32)
    nc.vector.tensor_scalar(t_b[:], p_cnt, 0.5, None, op0=OP.is_ge)
    t_bb = sb.tile([16, 1], f32)
    t_scrb = sb.tile([16, G], f32)
    nc.scalar.activation(t_scrb[:], t_b[:], AF.Copy,
                         accum_out=t_bb[:])
    t_scr = sb.tile([16, G], f32)
    t_bp = sb.tile([16, 1], f32)
    nc.vector.tensor_mul(t_scr[:], t_b[:], t_pp[:])
    nc.vector.tensor_reduce(t_bp[:], t_scr[:], axis=AX.X, op=OP.add)
    t_sp2 = sb.tile([16, 1], f32)
    t_scr2 = sb.tile([16, G], f32)
    nc.vector.scalar_tensor_tensor(
        t_scr2[:], t_pp[:], 1.0, t_pp[:],
        op0=OP.mult, op1=OP.mult, accum_out=t_sp2[:])
    t_rb = sb.tile([16, 1], f32)
    nc.vector.reciprocal(t_rb[:], t_bb[:])
    t_b2 = sb.tile([16, 1], f32)
    nc.vector.tensor_mul(t_b2[:], t_bp[:], t_bp[:])
    t_b3 = sb.tile([16, 1], f32)
    nc.vector.tensor_mul(t_b3[:], t_b2[:], t_rb[:])
    nc.vector.tensor_sub(t_fin[:, 0:1], t_sp2[:], t_b3[:])

    # ---- gx -> gq on ACT copy + Pool elementwise ----
    gxv = t_gxs[:].rearrange("p (s c) -> p s c", c=4)
    t_reg = sb.tile([16, 16], f32)
    t_img = sb.tile([16, 16], f32)
    nc.gpsimd.tensor_tensor(
        t_reg[:], gxv[:, :, 0], gxv[:, :, 3], op=OP.add)
    nc.gpsimd.tensor_tensor(
        t_img[:], gxv[:, :, 1], gxv[:, :, 2], op=OP.subtract)
    t_t1 = sb.tile([16, 16], f32)
    t_t2 = sb.tile([16, 16], f32)
    t_gq = sb.tile([16, 16], f32)
    nc.gpsimd.tensor_mul(t_t1[:], t_reg[:], t_reg[:])
    nc.gpsimd.tensor_mul(t_t2[:], t_img[:], t_img[:])
    nc.gpsimd.tensor_add(t_gq[:], t_t2[:], t_t1[:])

    # ---- downlink rates (nDU = 1e-9 dropped: < 1 ulp of den) ----
    # den[l,s] = nuu + sum_l' gq - gq; the broadcast sum comes from
    # a ones-matmul (every output partition gets the column sum).
    # ln(1+r) = ln(den+gq) - ln(den), den+gq = nuu + sum.
    t_onem = sb.tile([16, 16], f32)
    nc.vector.memset(t_onem[:], 1.0)
    p_den_b = ps.tile([16, 512], f32)
    p_den = p_den_b[:, 0:16]
    nc.tensor.matmul(p_den, t_onem[:], t_gq[:])
    t_q1 = sb.tile([16, 16], f32)
    nc.vector.scalar_tensor_tensor(
        t_q1[:], t_gq[:], -1.0, p_den, op0=OP.mult, op1=OP.add)
    t_den = sb.tile([16, 16], f32)
    nc.vector.tensor_add(t_den[:], t_q1[:], p_nu)
    t_dg = sb.tile([16, 16], f32)
    nc.vector.tensor_add(t_dg[:], t_den[:], t_gq[:])
    t_lnd = sb.tile([16, 16], f32)
    nc.scalar.activation(t_lnd[:], t_den[:], AF.Ln)
    t_lng = sb.tile([16, 16], f32)
    nc.scalar.activation(t_lng[:], t_dg[:], AF.Ln)
    t_lnr = sb.tile([16, 16], f32)
    nc.vector.scalar_tensor_tensor(
        t_lnr[:], t_lng[:], 1.0, t_lnd[:],
        op0=OP.mult, op1=OP.subtract, accum_out=t_fin[:, 1:2])

    # ---- store per-sample partials; host sums the 16 rows ----
    nc.sync.dma_start(d["out"][:], t_fin[:])




def _declare_drams(nc, mybir, suffix=""):
    f32 = mybir.dt.float32
    return {
        "rows0": nc.dram_tensor("rows0" + suffix, [64, ROWS_W // 2], f32,
                                kind="ExternalInput"),
        "rows1": nc.dram_tensor("rows1" + suffix, [64, ROWS_W // 2], f32,
                                kind="ExternalInput"),
        "agt": nc.dram_tensor("agt" + suffix, [64, AG_W], f32,
                              kind="ExternalInput"),
        "b128": nc.dram_tensor("b128" + suffix, [128, 17], f32,
                               kind="ExternalInput"),
        "dumt": nc.dram_tensor("dumt" + suffix, [64, DUMT_W], f32,
                               kind="ExternalInput"),
        "b32": nc.dram_tensor("b32" + suffix, [32, 272], f32,
                              kind="ExternalInput"),
        "out": nc.dram_tensor("out" + suffix, [16, 2], f32,
                              kind="ExternalOutput"),
    }


def _build_nc(replicas=1):
    import concourse.bass as bass
    import concourse.tile as tile
    from concourse import bacc, mybir

    nc = bacc.Bacc("TRN2", target_bir_lowering=False, debug=False)
    d = _declare_drams(nc, mybir)
    with tile.TileContext(nc) as tc:
        with (
            tc.tile_pool(name="sb", bufs=1) as sb,
            tc.tile_pool(name="ps", bufs=1, space=bass.MemorySpace.PSUM) as ps,
        ):
            for r in range(replicas):
                _emit_body(nc, tc, sb, ps, d, mybir, warm=(r == 0))
    nc.compile()
    return nc


def _host_prep(inputs):
    DUCom = np.asarray(inputs["DUComMat"])      # (B,L,NT) c64
    Sens = np.asarray(inputs["SensingMat"])     # (B,M,NT) c64
    DUMat = np.asarray(inputs["DUMatInit"])     # (B,L,NT) c64
    TAMat = np.asarray(inputs["TAMatInit"])     # (B,M,2) c64
    CI = np.asarray(inputs["CIMatInit"])        # (B,K,L) c64
    P = np.asarray(inputs["UUPowerMat"])        # (B,K) f32

    agT = _steering_consts()                    # (64, 2G)
    blk = np.zeros((128, 16), np.float32)
    for s in range(16):
        blk[8 * s:8 * s + 8, s] = 1.0

    in_maps = []
    for c in range(NCORES):
        gs = slice(c * S, (c + 1) * S)
        r = np.concatenate([DUCom[gs], Sens[gs]], axis=1)       # (S,24,64)
        re_t = np.transpose(r.real, (2, 0, 1))                  # (64,S,24)
        im_t = np.transpose(r.imag, (2, 0, 1))
        rows = np.stack([re_t, im_t], axis=2).reshape(64, ROWS_W)

        d = DUMat[gs]                                           # (S,L,64)
        dm = np.concatenate(
            [np.transpose(d.real, (2, 0, 1)),                   # (64,S,16)
             np.transpose(d.imag, (2, 0, 1))], axis=2
        ).reshape(64, DUMT_W)

        ci = CI[gs]                                             # (S,16,16)
        b32 = np.zeros((32, 272), np.float32)
        for s in range(S):
            j, cc = divmod(s, 2)
            r0 = 16 * cc
            b32[r0:r0 + 16, 32 * j:32 * j + 16] = ci[s].real
            b32[r0:r0 + 16, 32 * j + 16:32 * j + 32] = ci[s].imag
            b32[r0:r0 + 16, 256 + s] = P[gs][s]

        # col 0 = -TAang: the device computes |grid - ta| as Abs(grid + bias)
        b128 = np.concatenate(
            [-TAMat[gs][:, :, 0].real.reshape(128, 1).astype(np.float32),
             blk], axis=1)

        in_maps.append({
            "rows0": np.ascontiguousarray(rows[:, :ROWS_W // 2], np.float32),
            "rows1": np.ascontiguousarray(rows[:, ROWS_W // 2:], np.float32),
            "agt": agT,
            "b128": np.ascontiguousarray(b128, np.float32),
            "dumt": np.ascontiguousarray(dm, np.float32),
            "b32": np.ascontiguousarray(b32, np.float32),
        })
    return in_maps


def kernel(**inputs):
    from concourse.bass_utils import run_bass_kernel_spmd

    if "nc" not in _CACHE:
        _CACHE["nc"] = _build_nc()
    nc = _CACHE["nc"]

    in_maps = _host_prep(inputs)
    res = run_bass_kernel_spmd(nc, in_maps, core_ids=list(range(NCORES)))
    parts = np.array([res.results[c]["out"] for c in range(NCORES)],
                     dtype=np.float64)                           # (8,16,2)
    sd2 = parts[:, :, 0].sum()
    srln = parts[:, :, 1].sum()
    loss = 100.0 * sd2 / (G * B) - srln / (B * LN2) - 16.0
    return np.float32(loss)



# revision 53
# speedup vs baseline: 1.4942x; 1.4942x over previous
"""Trainium2 Bass kernel for nn_LossFunction_29145648071076.

Math notes (verified against the reference in float64 and in the numpy
emulation check_numerics.py):

  * Q = x x^H is rank-1 (x = sum of comm + sensing beams), so
      gHQg[b,l]  = |DUMatInit[b,l]^H x_b|^2,  P[b,g] = |a_g^H x_b|^2
    and no NTxNT matrices are ever needed.
  * The uplink MMSE path collapses exactly (Woodbury): sum_rate_uu = K
    = 16 to within 1e-7 bits; the kernel uses the constant.
  * nDU = 1e-9 on a ~21 denominator is below one f32 ulp; dropped.
  * sum diff^2 == sum P^2 - (sum bP)^2 / sum b   (b in {0,1}).
  * sum ln(1+r) = ln( prod(dg) / prod(den) ); the device ships the
    per-l row products (~1e21 << f32 max) and the host takes the 256
    final ln's during the gather.

Device layout (per core, S=16 samples):
  * complex products realized as 128-partition contractions:
    [re; im] stacks against xcat=[xr;xi] -> Re/Im straight off the PE.
  * steering tables ship from the host as bf16 in DMA1 (an on-device
    Sin pipeline was tried and lost: the act-table load plus the
    6-hop cross-engine chain costs more than the extra DMA bytes).
  * beampattern grid splits 128+53 on partitions; the 53-row chunk-B
    matmul outputs land in full-width PSUM columns that are zeroed by
    early memsets, so both chunks pack into single (128,32)/(128,96)
    elementwise ops and one (1,96) column-sum matmul.
  * output leaves via a prepare_only dma_scatter_add + trigger_dma
    (identity indices, zero-initialized destination), which skips the
    HWDGE (625ns) + DGE (650ns) serial latency of a normal store:
    descriptors are generated during the input-DMA dead time and the
    trigger is gated on the final writers' engine-completion ticks.
"""

import numpy as np

import ml_dtypes

B, NT, NR, K, L, M, I = 128, 64, 64, 16, 16, 8, 8
NCORES = 8
S = B // NCORES          # samples per core
G = 181                  # beampattern grid points
G2 = 256                 # padded grid (pad cols provably all-zero)
LN2 = float(np.log(2.0))
OUT_W = 128              # out cols: scanDG(16) scanDen(16) psS(96)
OUT_P = 128              # kv_writeback requires d_head % 128 == 0

# in1 (u16 cols): rows | ang(f32) | blk | agA | agB | scatter-idxs
C_ROWS, C_ANG, C_BLK, C_AGA, C_AGB, C_IDX, IN1_W = (
    0, 384, 386, 402, 583, 764, 772)
# in2 (u16 cols): ci | pw(f32) | dmA | dmB
C_CI, C_PW, C_DMA, C_DMB, IN2_W = 0, 64, 96, 352, 608

NWARM = 7
_CACHE = {}


def _steering_tables():
    """agA=[ar;ai], agB=[-ai;ar] (128, G) bf16, reference f32 rounding."""
    import ml_dtypes as mld
    grid = np.linspace(0.0, 180.0, G).astype(np.float32)
    n = np.arange(NT, dtype=np.float32)
    sin_t = np.sin(grid * np.float32(np.pi / 180.0)).astype(np.float32)
    phase = (np.float32(np.pi) * sin_t)[:, None] * n          # (G, NT)
    ar = np.cos(phase).astype(np.float32).T                   # (NT, G)
    ai = np.sin(phase).astype(np.float32).T
    agA = np.concatenate([ar, ai], axis=0)                    # (128, G)
    agB = np.concatenate([-ai, ar], axis=0)
    return agA.astype(mld.bfloat16), agB.astype(mld.bfloat16)


def _emit_body(nc, tc, sb, ps, d, mybir):
    AF = mybir.ActivationFunctionType
    OP = mybir.AluOpType
    AX = mybir.AxisListType
    f32 = mybir.dt.float32
    f32r = mybir.dt.float32r
    bf16 = mybir.dt.bfloat16
    u16 = mybir.dt.uint16
    i16 = mybir.dt.int16

    def r32(ap):
        return ap.bitcast(f32r)

    # Dummy Square: loads the small act table (Square/Copy serve every
    # ACT op in this kernel) while ACT is idle at kernel start.
    t_dl = sb.tile([1, 1], f32)
    nc.vector.memset(t_dl[:], 0.0)
    nc.scalar.activation(t_dl[:], t_dl[:], AF.Square)

    # ---- input DMAs (2 merged loads) ----
    t_in1 = sb.tile([128, IN1_W], u16)
    nc.sync.dma_start(t_in1[:], d["in1"][:])
    t_in2 = sb.tile([128, IN2_W], u16)
    nc.sync.dma_start(t_in2[:], d["in2"][:])

    rows = t_in1[:, C_ROWS:C_ANG].bitcast(bf16)
    ang = t_in1[:, C_ANG:C_BLK].bitcast(f32)       # (128,1) = -TAang
    blkb = t_in1[:, C_BLK:C_AGA].bitcast(bf16)     # (128,16) 0/1
    agA = t_in1[:, C_AGA:C_AGB].bitcast(bf16)      # (128,181)
    agB = t_in1[:, C_AGB:IN1_W].bitcast(bf16)
    cib = t_in2[:, C_CI:C_PW].bitcast(bf16)        # (128,64)
    pwf = t_in2[:, C_PW:C_PW + 16].bitcast(bf16)   # (128,16) sparse
    dmA = t_in2[:, C_DMA:C_DMB].bitcast(bf16)
    dmB = t_in2[:, C_DMB:IN2_W].bitcast(bf16)

    # ---- output tile + zero-init of the scatter destination ----
    t_fin = sb.tile([OUT_P, OUT_W], f32)
    nc.vector.memset(t_fin[:], 0.0)
    t_zero = sb.tile([OUT_P, OUT_W], f32)
    nc.vector.memset(t_zero[:], 0.0)
    nc.sync.dma_start(d["out"][:], t_zero[:])
    kv_sem = nc.alloc_semaphore("out_dma")

    # ---- constants ----
    t_ones = sb.tile([128, 1], f32)
    nc.vector.memset(t_ones[:], 1.0)
    t_ones16 = sb.tile([16, 16], bf16)
    nc.vector.memset(t_ones16[:], 1.0)
    t_grid = sb.tile([128, G], f32)
    nc.gpsimd.iota(t_grid[:], [[1, G]], channel_multiplier=0,
                   allow_small_or_imprecise_dtypes=True)

    # ---- PSUM tiles; garbage rows of the 53-row chunk-B matmul
    # outputs are zeroed up front so packed elementwise ops stay exact
    psAB = ps.tile([128, 64], f32)     # [ReA|ReB|ImA|ImB]
    psC = ps.tile([128, 32], f32)      # [cntA|cntB]
    psG = ps.tile([16, 32], f32)       # [Reg|Img]
    psN = ps.tile([16, 16], f32)       # nuu
    psCS = ps.tile([16, 16], f32)      # colsum(gq)
    psS = ps.tile([1, 96], f32)        # [uA uB vA vB bA bB] col sums
    nc.vector.memset(psAB[:, 16:32], 0.0)
    nc.vector.memset(psAB[:, 48:64], 0.0)
    nc.vector.memset(psC[:, 16:32], 0.0)


    # ---- x = row sums -> xcat (128,16) bf16 = [xr; xi] ----
    t_xb = sb.tile([128, S], bf16)
    rows_v = rows.rearrange("p (s j) -> p s j", j=24)
    with nc.allow_low_precision(reason="x fits bf16; consumers are bf16 matmuls"):
        nc.vector.tensor_reduce(t_xb[:], rows_v[:], axis=AX.X, op=OP.add)

    # ---- mask: d = |grid + ang| (ACT Abs, per-partition bias),
    # ind = d <= 10 (DVE) -- the baseline-proven op pair ----
    t_d = sb.tile([128, G], f32)
    nc.scalar.activation(t_d[:], t_grid[:], AF.Abs, bias=ang)
    t_ind = sb.tile([128, G], bf16)
    nc.vector.tensor_scalar(t_ind[:], t_d[:], 10.0, None, op0=OP.is_le)

    # ---- identity scatter idxs arrive with DMA1 (constant table) ----
    t_idx = t_in1[:, C_IDX:IN1_W].bitcast(i16)
    fin_v = t_fin[:].rearrange("p (j e) -> p j e", j=1)
    nc.gpsimd.dma_scatter_add(d["out"][:], fin_v, t_idx, OUT_P, OUT_P,
                              OUT_W, prepare_only=True, sem=kv_sem)


    # ---- |CI|^2 (ACT); re/im fold happens inside the nuu matmuls ----
    t_sq = sb.tile([128, 64], bf16)
    nc.scalar.activation(t_sq[:], cib, AF.Square)

    # ---- PE ----
    nc.tensor.matmul(psAB[:, 0:16], agA[:, 0:128], t_xb[:])
    nc.tensor.matmul(psAB[0:53, 16:32], agA[:, 128:G], t_xb[:])
    nc.tensor.matmul(psAB[:, 32:48], agB[:, 0:128], t_xb[:])
    nc.tensor.matmul(psAB[0:53, 48:64], agB[:, 128:G], t_xb[:])
    for s in range(S):
        nc.tensor.matmul(psG[:, s:s + 1], dmA[:, 16 * s:16 * s + 16],
                         t_xb[:, s:s + 1])
        nc.tensor.matmul(psG[:, 16 + s:17 + s], dmB[:, 16 * s:16 * s + 16],
                         t_xb[:, s:s + 1])
    nc.tensor.matmul(psC[:, 0:16], t_ind[:, 0:128], blkb)
    nc.tensor.matmul(psC[0:53, 16:32], t_ind[:, 128:G], blkb)
    nc.tensor.matmul(psN[:, 0:8], t_sq[:, 0:16], pwf[:, 0:8],
                     start=True, stop=False)
    nc.tensor.matmul(psN[:, 0:8], t_sq[:, 16:32], pwf[:, 0:8],
                     start=False, stop=True)
    nc.tensor.matmul(psN[:, 8:16], t_sq[:, 32:48], pwf[:, 8:16],
                     start=True, stop=False)
    nc.tensor.matmul(psN[:, 8:16], t_sq[:, 48:64], pwf[:, 8:16],
                     start=False, stop=True)

    # ---- elementwise tail; DVE ops emitted in dependency-depth order
    # (s1, g1, pa, gq, v, dg) so its serial queue tracks the frontier ----
    # HW: an ALU op may read at most one PSUM input, so squares of PSUM
    # tensors go through ACT or an SBUF staging copy.
    t_s1 = sb.tile([128, 32], f32)
    nc.scalar.activation(t_s1[:], psAB[:, 0:32], AF.Square)
    t_gc = sb.tile([16, 32], f32)
    nc.vector.tensor_copy(t_gc[:], psG[:])
    t_s2 = sb.tile([128, 32], f32)
    nc.scalar.activation(t_s2[:], psAB[:, 32:64], AF.Square)
    t_g1 = sb.tile([16, 16], f32)
    nc.vector.tensor_tensor(t_g1[:], t_gc[:, 0:16], t_gc[:, 0:16],
                            op=OP.mult)
    t_pa = sb.tile([128, 32], f32)
    nc.vector.tensor_tensor(t_pa[:], t_s1[:], t_s2[:], op=OP.add)
    t_g2 = sb.tile([16, 16], f32)
    nc.vector.tensor_tensor(t_g2[:], t_gc[:, 16:32], t_gc[:, 16:32],
                            op=OP.mult)
    gq_out = t_fin[0:16, 16:32]
    nc.vector.tensor_tensor(gq_out, t_g1[:], t_g2[:], op=OP.add)
    t_gqb = sb.tile([16, 16], bf16)
    nc.vector.tensor_copy(t_gqb[:], gq_out)
    nc.tensor.matmul(psCS[:], t_ones16[:], t_gqb[:])
    t_w = sb.tile([128, 96], f32)      # [u | v | b]
    nc.vector.tensor_scalar(t_w[:, 64:96], psC[:], 0.5, None, op0=OP.is_ge)
    nc.scalar.activation(t_w[:, 0:32], t_pa[:], AF.Square)
    nc.vector.tensor_tensor(t_w[:, 32:64], t_w[:, 64:96], t_pa[:],
                            op=OP.mult)
    t_cs = sb.tile([16, 16], f32)
    nc.vector.tensor_copy(t_cs[:], psCS[:])
    w_dg = nc.vector.tensor_tensor(t_fin[0:16, 0:16], t_cs[:], psN[:],
                                   op=OP.add)
    nc.tensor.matmul(psS[:], t_ones[:], t_w[:])
    # beampattern column sums -> row 0 of the out tile
    w_ps = nc.scalar.activation(t_fin[0:1, 32:128], psS[:], AF.Copy)

    d["_writers"] = [w_dg.ins.name, w_ps.ins.name]
    nc.gpsimd.trigger_dma(count=None)
    # completion wait on SP: a Pool-side wait would hold Pool SEQ, which
    # the trigger's transfer track needs to re-acquire (deadlock).
    nc.sync.wait_ge(kv_sem, 16)


def _declare_drams(nc, mybir):
    f32 = mybir.dt.float32
    u16 = mybir.dt.uint16
    return {
        "in1": nc.dram_tensor("in1", [128, IN1_W], u16,
                              kind="ExternalInput"),
        "in2": nc.dram_tensor("in2", [128, IN2_W], u16,
                              kind="ExternalInput"),
        "out": nc.dram_tensor("out", [OUT_P, OUT_W], f32,
                              kind="ExternalOutput"),
    }


def _build_nc():
    import concourse.bass as bass
    import concourse.tile as tile
    from concourse import bacc, mybir

    nc = bacc.Bacc("TRN2", target_bir_lowering=False, debug=False)
    d = _declare_drams(nc, mybir)
    with tile.TileContext(nc) as tc:
        with (
            tc.tile_pool(name="sb", bufs=1) as sb,
            tc.tile_pool(name="ps", bufs=1, space=bass.MemorySpace.PSUM) as ps,
        ):
            _emit_body(nc, tc, sb, ps, d, mybir)
    _patch_trigger_waits(nc, mybir, d["_writers"])
    _patch_dmasw_epilogue(nc, mybir)
    nc.compile()
    return nc


def _patch_trigger_waits(nc, mybir, writer_names):
    """Gate trigger_dma on the engine-completion ticks of the final
    out-tile writers (post-scheduling, so queue positions are fixed).
    Tile's engine-lane sems increment once per non-sequencer instruction
    in scheduled order; wait value = the writer's 1-based count on its
    engine (calibrated against waits Tile itself emits)."""
    insts = [i for b in nc.m.functions[0].blocks for i in b.instructions]
    cnt = {}
    tick = {}
    for i in insts:
        try:
            seq_only = i.is_sequencer_only()
        except Exception:
            seq_only = True
        if not seq_only:
            e = str(i.engine)
            cnt[e] = cnt.get(e, 0) + 1
            tick[i.name] = (i.engine, cnt[e])
    lane = {}
    for i in insts:
        si = i.sync_info
        if si and si.on_wait:
            for w in si.on_wait:
                if w.ant_name:
                    lane[w.ant_name] = w.id
    eng_lane = {}
    for name, sid in lane.items():
        for pref in ("DVE_", "Activation_", "Pool_", "PE_", "SP_"):
            if name.startswith(pref):
                eng_lane[pref[:-1]] = (sid, name)
    trig = next(i for i in insts if type(i).__name__ == "InstTriggerDma")
    ws = list(trig.sync_info.on_wait or [])
    for wn in writer_names:
        eng, t = tick[wn]
        key = str(eng).split(".")[-1]
        sid, sname = eng_lane[key]
        ws.append(mybir.SyncWait(
            sync_type='semaphore', id=sid, ant_name=sname,
            wait_mode='sem-ge-imm', wait_value=t))
    trig.sync_info.on_wait = ws


def _patch_trigger_waits(nc, mybir, writer_names):
    """Gate trigger_dma on the engine-completion ticks of the final
    out-tile writers.  Tile's engine-lane sems increment once per
    non-sequencer instruction, in scheduled order; the wait value for a
    writer is its 1-based count on its engine (calibrated against the
    waits Tile itself emits)."""
    insts = [i for b in nc.m.functions[0].blocks for i in b.instructions]
    cnt = {}
    tick = {}
    for i in insts:
        try:
            seq_only = i.is_sequencer_only()
        except Exception:
            seq_only = True
        if not seq_only:
            e = str(i.engine)
            cnt[e] = cnt.get(e, 0) + 1
            tick[i.name] = (i.engine, cnt[e])
    lane = {}
    for i in insts:
        si = i.sync_info
        if si and si.on_wait:
            for w in si.on_wait:
                if w.ant_name:
                    lane[w.ant_name] = w.id
    eng_lane = {}
    for name, sid in lane.items():
        for pref in ("DVE_", "Activation_", "Pool_", "PE_", "SP_"):
            if name.startswith(pref):
                eng_lane[pref[:-1]] = (sid, name)
    trig = next(i for i in insts if type(i).__name__ == "InstTriggerDma")
    ws = list(trig.sync_info.on_wait or [])
    for wn in writer_names:
        eng, t = tick[wn]
        key = str(eng).split(".")[-1]
        sid, sname = eng_lane[key]
        ws.append(mybir.SyncWait(
            sync_type='semaphore', id=sid, ant_name=sname,
            wait_mode='sem-ge-imm', wait_value=t))
    trig.sync_info.on_wait = ws


def _patch_dmasw_epilogue(nc, mybir):
    """Tile's epilogue waits on its DMASW lane sem for the prepare_only
    writeback, but the descriptor's single completion-sem slot carries
    the user sem (kvwb_dma), so the lane sem never moves.  Point the
    epilogue wait at the user sem instead: same value (16), same
    DMA-completion event."""
    insts = [i for b in nc.m.functions[0].blocks for i in b.instructions]
    prep = next(i for i in insts
                if type(i).__name__ == "InstDMAScatterAddAnt")
    kv_upd = prep.sync_info.on_update[0]
    assert kv_upd.ant_name == "out_dma"
    updated_ids = set()
    for i in insts:
        si = i.sync_info
        if si and si.on_update:
            for u in si.on_update:
                updated_ids.add(u.id)
    for i in insts:
        si = i.sync_info
        if not si or not si.on_wait:
            continue
        ws = list(si.on_wait)
        changed = False
        for k, w in enumerate(ws):
            if (w.ant_name and w.ant_name.startswith("DMASW")
                    and w.id not in updated_ids):
                ws[k] = mybir.SyncWait(
                    sync_type='semaphore', id=kv_upd.id,
                    ant_name=kv_upd.ant_name, wait_mode=w.wait_mode,
                    wait_value=int(w.wait_value))
                changed = True
        if changed:
            si.on_wait = ws


def _host_prep(inputs):
    DUCom = np.asarray(inputs["DUComMat"])      # (B,L,NT) c64
    Sens = np.asarray(inputs["SensingMat"])     # (B,M,NT) c64
    DUMat = np.asarray(inputs["DUMatInit"])     # (B,L,NT) c64
    TAMat = np.asarray(inputs["TAMatInit"])     # (B,M,2) c64
    CI = np.asarray(inputs["CIMatInit"])        # (B,K,L) c64
    Pw = np.asarray(inputs["UUPowerMat"])       # (B,K) f32

    agA, agB = _steering_tables()
    blk = np.zeros((128, 16), ml_dtypes.bfloat16)
    for s in range(16):
        blk[8 * s:8 * s + 8, s] = 1.0

    in_maps = []
    for c in range(NCORES):
        gs = slice(c * S, (c + 1) * S)
        r = np.concatenate([DUCom[gs], Sens[gs]], axis=1)       # (S,24,64)
        rows = np.concatenate(
            [np.transpose(r.real, (2, 0, 1)).reshape(64, 384),
             np.transpose(r.imag, (2, 0, 1)).reshape(64, 384)], axis=0)

        ang = -TAMat[gs][:, :, 0].real.reshape(128, 1).astype(np.float32)

        ci = CI[gs]                                             # (S,16,16)
        cit = np.zeros((128, 64), np.float32)
        pw = np.zeros((128, 16), np.float32)
        for so in range(8):
            rr = slice(16 * so, 16 * so + 16)
            cit[rr, 0:16] = ci[so].real
            cit[rr, 16:32] = ci[so].imag
            cit[rr, 32:48] = ci[so + 8].real
            cit[rr, 48:64] = ci[so + 8].imag
            pw[rr, so] = Pw[gs][so]
            pw[rr, 8 + so] = Pw[gs][so + 8]

        in1 = np.zeros((128, IN1_W), np.uint16)
        in1[:, C_ROWS:C_ANG] = rows.astype(ml_dtypes.bfloat16).view(np.uint16)
        in1[:, C_ANG:C_BLK] = ang.view(np.uint16).reshape(128, 2)
        in1[:, C_BLK:C_AGA] = blk.view(np.uint16)

        dm = DUMat[gs]                                          # (S,L,64)
        dmr = np.transpose(dm.real, (2, 0, 1)).reshape(64, 256)
        dmi = np.transpose(dm.imag, (2, 0, 1)).reshape(64, 256)
        in1[:, C_AGA:C_AGB] = agA.view(np.uint16)
        in1[:, C_AGB:C_IDX] = agB.view(np.uint16)
        idxs = (np.arange(128)[:, None] % 16
                + 16 * np.arange(8)[None, :]).astype(np.int16)
        in1[:, C_IDX:IN1_W] = idxs.view(np.uint16)
        in2 = np.zeros((128, IN2_W), np.uint16)
        in2[:, C_CI:C_PW] = cit.astype(ml_dtypes.bfloat16).view(np.uint16)
        in2[:, C_PW:C_PW + 16] = pw.astype(
            ml_dtypes.bfloat16).view(np.uint16)
        in2[:, C_DMA:C_DMB] = np.concatenate(
            [dmr, dmi], axis=0).astype(ml_dtypes.bfloat16).view(np.uint16)
        in2[:, C_DMB:IN2_W] = np.concatenate(
            [-dmi, dmr], axis=0).astype(ml_dtypes.bfloat16).view(np.uint16)

        in_maps.append({"in1": in1, "in2": in2})
    return in_maps


def _postprocess(parts):
    """parts: list of 8 (16, OUT_W) f32 arrays -> scalar loss."""
    sd2 = 0.0
    srln = 0.0
    for out in parts:
        o = np.asarray(out, np.float64)
        dg = o[0:16, 0:16]
        gq = o[0:16, 16:32]
        den = dg - gq
        srln += float(np.log(dg / den).sum())
        Ssum = o[0, 32:128].reshape(6, 16)
        sumU = Ssum[0] + Ssum[1]
        sumV = Ssum[2] + Ssum[3]
        sumB = Ssum[4] + Ssum[5]
        sd2 += float((sumU - sumV * sumV / sumB).sum())
    return np.float32(100.0 * sd2 / (G * B) - srln / (B * LN2) - 16.0)


def kernel(**inputs):
    from concourse.bass_utils import run_bass_kernel_spmd

    if "nc" not in _CACHE:
        _CACHE["nc"] = _build_nc()
    nc = _CACHE["nc"]

    in_maps = _host_prep(inputs)
    res = run_bass_kernel_spmd(nc, in_maps, core_ids=list(range(NCORES)))
    return _postprocess([res.results[c]["out"] for c in range(NCORES)])
